# revision 1
# baseline (speedup 1.0000x reference)
"""DGCNN (4x EdgeConv + final projection + global max) on 8 Trainium2 cores.

Sharding: data-parallel over batch B=8 -> one point cloud per NeuronCore.

Per-core algorithm (N=2048 points, k=40 neighbors):
  Each EdgeConv layer `h' = max_k lrelu(concat(h_j - h_i, h_i) @ W + b)` is
  algebraically decomposed (lrelu monotone, V_i constant over neighbors j):
      U = h @ W_top          (N, D)
      V = h @ (W_bot - W_top) + b
      h'[i] = lrelu(max_{j in knn(i)} U[j] + V[i])
  so no (N, k, 2C) edge tensor is ever materialized.

  kNN scores are the recentred s[i,j] = h_i.h_j - |h_j|^2/2 - |h_i|^2/2
  = -dist(i,j)/2 <= 0, computed on the PE in fp16 (main product + hi/lo
  split fp16 rank-1 updates for the norm terms so the large norm values
  don't lose the small distance differences). Top-40 per 128-row tile on
  the DVE via 5 rounds of max8/max_index/match_replace on fp16 scores.
  The neighbor rows of U (stored fp16 in HBM) are fetched with ONE
  batched indirect DMA per tile (5120 descriptors) instead of 40 -- the
  per-instruction SWDGE overhead (~1us) made 2560 small gathers the
  dominant cost of the previous version.
"""

import numpy as np

import concourse.mybir as mybir
import concourse.tile as tile
from concourse import bass_utils, library_config
from concourse.bacc import Bacc
from concourse.masks import make_identity

FP32 = mybir.dt.float32
FP16 = mybir.dt.float16
U16 = mybir.dt.uint16
I16 = mybir.dt.int16

# Problem constants (hardcoded per harness contract)
B = 8
N = 2048
IN_CHAN = 3
H_DIM = [64, 64, 128, 256]
Z_DIM = 512
K = 40
N_CORES = 8


def build_program(n=N, k=K, in_chan=IN_CHAN, h_dim=None, z2=2 * Z_DIM):
    """Build the single-core Bacc program."""
    h_dim = h_dim or H_DIM
    nt = n // 128            # row tiles
    nfb = n // 512           # 512-wide free blocks
    rounds = k // 8
    dmax = max(h_dim)

    nc = Bacc("TRN2", target_bir_lowering=False, debug=False,
              num_devices=N_CORES, num_swdge_queues=4)

    # ---------------- DRAM tensors ----------------
    xT = nc.dram_tensor("xT", [in_chan, n], FP16, kind="ExternalInput")
    xR = nc.dram_tensor("xR", [n, in_chan], FP16, kind="ExternalInput")
    uw_d, vw_d, vb_d = [], [], []
    cins = [in_chan] + [h for h in h_dim[:-1]]
    for l in range(4):
        c, d = cins[l], h_dim[l]
        uw_d.append(nc.dram_tensor(f"uw{l}", [c, d], FP16, kind="ExternalInput"))
        vw_d.append(nc.dram_tensor(f"vw{l}", [c, d], FP16, kind="ExternalInput"))
        vb_d.append(nc.dram_tensor(f"vb{l}", [1, d], FP32, kind="ExternalInput"))
    # final weight chunks: rows split as [h1, h2, h3, h4(128-chunks)]
    wf_chunk_rows = []
    acc = 0
    for l in range(4):
        d = h_dim[l]
        off = 0
        while off < d:
            rows = min(128, d - off)
            wf_chunk_rows.append((l, off, rows, acc))
            acc += rows
            off += rows
    wf_d = [nc.dram_tensor(f"wf{i}", [rows, z2], FP16, kind="ExternalInput")
            for i, (_, _, rows, _) in enumerate(wf_chunk_rows)]

    # U rows must be a multiple of 256B for dma_gather: pad d=64 to 128 fp16
    u_pad = [max(h_dim[l], 128) for l in range(4)]
    u_dram = [nc.dram_tensor(f"u_scratch{l}", [n, u_pad[l]], FP16,
                             kind="Internal") for l in range(4)]
    # per-tile neighbor-index scratch for the wrap relayout (double buffered)
    iscr_dram = [nc.dram_tensor(f"iscr{j}", [128, k], FP16, kind="Internal")
                 for j in range(2)]
    repmat_d = nc.dram_tensor("repmat", [16, 128], FP16, kind="ExternalInput")
    out_dram = nc.dram_tensor("out", [128, z2 // 128], FP32, kind="ExternalOutput")

    amax = mybir.AluOpType.max

    with tile.TileContext(nc) as tc:
        with tc.tile_pool(name="pers", bufs=1) as pers, \
             tc.tile_pool(name="sbuf", bufs=2) as sb, \
             tc.tile_pool(name="ps_s", bufs=2, space="PSUM") as ps_s, \
             tc.tile_pool(name="ps_m", bufs=2, space="PSUM") as ps_m:

            # ------------- persistent SBUF -------------
            hT = [pers.tile([max(c, 1), n], FP16, tag=f"hT{l}", name=f"hT{l}")
                  for l, c in enumerate(cins)]
            h4 = [pers.tile([128, n], FP16, tag=f"h4_{j}", name=f"h4_{j}")
                  for j in range(dmax // 128)]
            ones16v = pers.tile([1, 128], FP16, tag="o16v", name="o16v")
            ones16f = pers.tile([1, 512], FP16, tag="o16f", name="o16f")
            ones32 = pers.tile([1, 128], FP32, tag="o32", name="o32")
            onescol32 = pers.tile([128, 1], FP32, tag="oc32", name="oc32")
            ident16 = pers.tile([128, 128], FP16, tag="id16", name="id16")
            sqneg32 = pers.tile([1, n], FP32, tag="sqn", name="sqn")
            lo32 = pers.tile([1, n], FP32, tag="lo32", name="lo32")
            hi16 = pers.tile([1, n], FP16, tag="hi16", name="hi16")
            lo16 = pers.tile([1, n], FP16, tag="lo16", name="lo16")
            idx = pers.tile([128, nt, k], U16, tag="idx", name="idx")
            repmat = pers.tile([16, 128], FP16, tag="repmat", name="repmat")
            x_r = pers.tile([128, nt, in_chan], FP16, tag="x_r", name="x_r")
            sq_r = pers.tile([128, nt], FP32, tag="sq_r", name="sq_r")
            sq_scr = pers.tile([128, dmax], FP32, tag="sq_scr", name="sq_scr")
            m_sb = pers.tile([128, nt, dmax], FP16, tag="m", name="m")
            v_sb = pers.tile([128, nt, dmax], FP16, tag="v", name="v")
            uw = [pers.tile([cins[l], h_dim[l]], FP16, tag=f"uw{l}", name=f"uw{l}")
                  for l in range(4)]
            vw = [pers.tile([cins[l], h_dim[l]], FP16, tag=f"vw{l}", name=f"vw{l}")
                  for l in range(4)]
            vb = [pers.tile([1, h_dim[l]], FP32, tag=f"vb{l}", name=f"vb{l}")
                  for l in range(4)]
            wf = [pers.tile([rows, z2], FP16, tag=f"wf{i}", name=f"wf{i}")
                  for i, (_, _, rows, _) in enumerate(wf_chunk_rows)]
            red = pers.tile([128, (z2 // 128) * nfb], FP32, tag="red", name="red")
            out_sb = pers.tile([128, z2 // 128], FP32, tag="out_sb", name="out_sb")

            # ------------- stage inputs -------------
            nc.gpsimd.load_library(library_config.mlp)
            nc.sync.dma_start(repmat[:], repmat_d.ap())
            nc.sync.dma_start(hT[0][:in_chan, :], xT.ap())
            nc.sync.dma_start(
                x_r[:], xR.ap().rearrange("(t p) c -> p t c", p=128))
            for l in range(4):
                nc.sync.dma_start(uw[l][:], uw_d[l].ap())
                nc.sync.dma_start(vw[l][:], vw_d[l].ap())
                nc.sync.dma_start(vb[l][:], vb_d[l].ap())
            for i in range(len(wf)):
                nc.sync.dma_start(wf[i][:], wf_d[i].ap())
            nc.gpsimd.memset(ones16v[:], 1.0)
            nc.gpsimd.memset(ones16f[:], 1.0)
            nc.gpsimd.memset(ones32[:], 1.0)
            nc.gpsimd.memset(onescol32[:], 1.0)
            make_identity(nc, ident16[:])

            # ------------- EdgeConv layers -------------
            for l in range(4):
                c, d = cins[l], h_dim[l]
                ht = hT[l][:c, :]

                # column norms: sqneg = -|h_j|^2/2 (fp32), split hi16+lo16
                l2sq = sb.tile([128, n], FP32, tag="l2sq", name="l2sq")
                nc.scalar.activation(l2sq[:c, :], ht,
                                     mybir.ActivationFunctionType.Square)
                for fb in range(nfb):
                    fs = slice(fb * 512, (fb + 1) * 512)
                    p_q = ps_m.tile([128, 512], FP32, tag="misc", name="misc")
                    nc.tensor.matmul(p_q[:1, :], lhsT=onescol32[:c, :],
                                     rhs=l2sq[:c, fs], start=True, stop=True)
                    nc.scalar.activation(sqneg32[:, fs], p_q[:1, :],
                                         mybir.ActivationFunctionType.Copy,
                                         scale=-0.5)
                nc.scalar.copy(hi16[:], sqneg32[:])
                nc.vector.tensor_tensor(out=lo32[:], in0=sqneg32[:],
                                        in1=hi16[:], op=mybir.AluOpType.subtract)
                nc.scalar.copy(lo16[:], lo32[:])

                # row norms -> sq_r = -|h_i|^2/2 (per-partition bias for the
                # score copy; replaces two PE rank-1 updates per block)
                for tb in range(nt):
                    if l == 0:
                        nc.scalar.activation(
                            sq_scr[:, :c], x_r[:, tb, :],
                            mybir.ActivationFunctionType.Square,
                            accum_out=sq_r[:, tb:tb + 1])
                    else:
                        nc.scalar.activation(
                            sq_scr[:, :c], m_sb[:, tb, :c],
                            mybir.ActivationFunctionType.Square,
                            accum_out=sq_r[:, tb:tb + 1])
                nc.vector.tensor_scalar_mul(sq_r[:], sq_r[:], -0.5)

                # U / V for every tile
                for tb in range(nt):
                    bs = slice(tb * 128, (tb + 1) * 128)
                    p_u = ps_m.tile([128, 512], FP32, tag="misc", name="misc")
                    nc.tensor.matmul(p_u[:, :d], lhsT=ht[:, bs], rhs=uw[l][:],
                                     start=True, stop=True)
                    ustage = sb.tile([128, dmax], FP16, tag="ustage",
                                     name="ustage")
                    nc.scalar.copy(ustage[:, :d], p_u[:, :d])
                    nc.sync.dma_start(
                        u_dram[l].ap().rearrange("(t p) d -> t p d",
                                                 p=128)[tb][:, :d],
                        ustage[:, :d])

                    p_v = ps_m.tile([128, 512], FP32, tag="misc", name="misc")
                    nc.tensor.matmul(p_v[:, :d], lhsT=ht[:, bs], rhs=vw[l][:],
                                     start=True, stop=False)
                    nc.tensor.matmul(p_v[:, :d], lhsT=ones32[:], rhs=vb[l][:],
                                     start=False, stop=True)
                    nc.scalar.copy(v_sb[:, tb, :d], p_v[:, :d])

                # scores + top-k per tile, then one batched gather + k-max.
                # The k-reduce of tile t is deferred past tile t+1's top-k so
                # the DVE never stalls on the gather DMA.
                pending = None
                for tb in range(nt):
                    bs = slice(tb * 128, (tb + 1) * 128)
                    s_sb = sb.tile([128, n], FP32, tag="s_sb", name="s_sb")
                    for fb in range(nfb):
                        fs = slice(fb * 512, (fb + 1) * 512)
                        p_s = ps_s.tile([128, 512], FP32, tag="s", name="s")
                        nc.tensor.matmul(p_s[:], lhsT=ht[:, bs],
                                         rhs=ht[:, fs], start=True, stop=False)
                        nc.tensor.matmul(p_s[:], lhsT=ones16v[:],
                                         rhs=hi16[:, fs], start=False, stop=False)
                        nc.tensor.matmul(p_s[:], lhsT=ones16v[:],
                                         rhs=lo16[:, fs], start=False, stop=True)
                        nc.scalar.activation(
                            s_sb[:, fs], p_s[:],
                            mybir.ActivationFunctionType.Identity,
                            bias=sq_r[:, tb:tb + 1])
                    for r in range(rounds):
                        vals8 = sb.tile([128, 8], FP32, tag="vals8", name="vals8")
                        nc.vector.max(out=vals8[:], in_=s_sb[:])
                        nc.vector.max_index(
                            out=idx[:, tb, 8 * r:8 * r + 8],
                            in_max=vals8[:], in_values=s_sb[:])
                        nc.vector.match_replace(
                            out=s_sb[:], in_to_replace=vals8[:],
                            in_values=s_sb[:], imm_value=-3.0e38)
                    # relayout idx [128, k] into dma_gather's 16-partition wrap
                    # (replicated to all 8 gpsimd cores via a 0/1 matmul):
                    # iwrap[q, 8t+j] = idx[16j+q, t]
                    jb = tb % 2
                    idx16 = sb.tile([128, k], FP16, tag="idx16", name="idx16")
                    nc.vector.tensor_copy(out=idx16[:], in_=idx[:, tb, :])
                    nc.sync.dma_start(iscr_dram[jb].ap(), idx16[:])
                    jm = sb.tile([16, 8, k], FP16, tag="jm", name="jm")
                    nc.sync.dma_start(
                        jm[:], iscr_dram[jb].ap().rearrange("(j q) t -> q j t",
                                                            q=16))
                    p_w = ps_m.tile([128, 8 * k], FP32, tag="wrap", name="wrap")
                    nc.tensor.matmul(p_w[:], lhsT=repmat[:],
                                     rhs=jm[:].rearrange("q j t -> q t j"),
                                     start=True, stop=True)
                    iwrap = sb.tile([128, 8 * k], I16, tag="iwrap", name="iwrap")
                    nc.vector.tensor_copy(out=iwrap[:], in_=p_w[:])
                    # 1024 descriptors (16KB carveout) per dma_gather
                    gdest = sb.tile([128, k, u_pad[l]], FP16,
                                    tag=f"gd{u_pad[l]}", name="gdest")
                    for ch in range(k // 8):
                        nc.gpsimd.dma_gather(
                            gdest[:, 8 * ch:8 * ch + 8, :], u_dram[l].ap(),
                            iwrap[:, 64 * ch:64 * ch + 64],
                            num_idxs=1024, num_idxs_reg=1024,
                            elem_size=u_pad[l], queue_num=ch % 4)
                    if pending is not None:
                        pt, pg = pending
                        nc.vector.tensor_reduce(
                            out=m_sb[:, pt, :d],
                            in_=pg[:, :, :d].rearrange("p k d -> p d k"),
                            axis=mybir.AxisListType.X, op=amax)
                    pending = (tb, gdest)
                pt, pg = pending
                nc.vector.tensor_reduce(
                    out=m_sb[:, pt, :d],
                    in_=pg[:, :, :d].rearrange("p k d -> p d k"),
                    axis=mybir.AxisListType.X, op=amax)

                # h' = lrelu(M + V)
                nc.vector.tensor_tensor(out=m_sb[:, :, :d], in0=m_sb[:, :, :d],
                                        in1=v_sb[:, :, :d], op=mybir.AluOpType.add)
                nc.vector.scalar_tensor_tensor(
                    out=m_sb[:, :, :d], in0=m_sb[:, :, :d], scalar=0.2,
                    in1=m_sb[:, :, :d], op0=mybir.AluOpType.mult, op1=amax)

                # transpose h' (N, d) -> next layer's (d, N)
                for tb in range(nt):
                    bs = slice(tb * 128, (tb + 1) * 128)
                    for dc in range((d + 127) // 128):
                        rows = min(128, d - dc * 128)
                        p_t = ps_m.tile([128, 128], FP16, tag="tr", name="tr")
                        nc.tensor.transpose(
                            p_t[:rows, :128],
                            in_=m_sb[:, tb, dc * 128:dc * 128 + rows],
                            identity=ident16[:])
                        if l < 3:
                            dst = hT[l + 1][dc * 128:dc * 128 + rows, bs]
                        else:
                            dst = h4[dc][:rows, bs]
                        nc.scalar.copy(dst, p_t[:rows, :128])

            # ------------- final projection + global max -------------
            h_bufs = {0: hT[1][:h_dim[0], :], 1: hT[2][:h_dim[1], :],
                      2: hT[3][:h_dim[2], :]}
            nmb = z2 // 128
            for mb in range(nmb):
                ms = slice(mb * 128, (mb + 1) * 128)
                for fb in range(nfb):
                    fs = slice(fb * 512, (fb + 1) * 512)
                    p_f = ps_s.tile([128, 512], FP32, tag="s", name="s")
                    for i, (l, off, rows, _) in enumerate(wf_chunk_rows):
                        if l < 3:
                            rhs = h_bufs[l][off:off + rows, fs]
                        else:
                            rhs = h4[off // 128][:rows, fs]
                        nc.tensor.matmul(p_f[:], lhsT=wf[i][:, ms], rhs=rhs,
                                         start=(i == 0),
                                         stop=(i == len(wf_chunk_rows) - 1))
                    nc.vector.tensor_reduce(
                        out=red[:, mb * nfb + fb:mb * nfb + fb + 1],
                        in_=p_f[:], axis=mybir.AxisListType.X, op=amax)
            nc.vector.tensor_reduce(
                out=out_sb[:],
                in_=red[:].rearrange("p (m f) -> p m f", f=nfb),
                axis=mybir.AxisListType.X, op=amax)
            nc.sync.dma_start(out_dram.ap(), out_sb[:])

    nc.compile()
    return nc


def _prep_core_inputs(x_c, params, h_dim, in_chan):
    """Host-side input prep for one core: transpose x, split/derive weights."""
    cins = [in_chan] + [h for h in h_dim[:-1]]
    m = {"xT": np.ascontiguousarray(x_c.T).astype(np.float16),
         "xR": np.ascontiguousarray(x_c).astype(np.float16)}
    m["repmat"] = (np.arange(128)[None, :] % 16 ==
                   np.arange(16)[:, None]).astype(np.float16)
    for l in range(4):
        c = cins[l]
        w = params[f"W{l}"]
        m[f"uw{l}"] = np.ascontiguousarray(w[:c]).astype(np.float16)
        m[f"vw{l}"] = np.ascontiguousarray(w[c:] - w[:c]).astype(np.float16)
        m[f"vb{l}"] = params[f"b{l}"][None, :].astype(np.float32)
    wfull = params["Wf"]
    i = 0
    acc = 0
    for l in range(4):
        d = h_dim[l]
        off = 0
        while off < d:
            rows = min(128, d - off)
            m[f"wf{i}"] = np.ascontiguousarray(wfull[acc:acc + rows]).astype(np.float16)
            acc += rows
            off += rows
            i += 1
    return m


_NC_CACHE = {}


def kernel(**inputs):
    x = np.asarray(inputs["x"], dtype=np.float32)
    params = {k_: np.asarray(v, dtype=np.float32) for k_, v in inputs.items()
              if k_ != "x"}

    if "nc" not in _NC_CACHE:
        _NC_CACHE["nc"] = build_program()
    nc = _NC_CACHE["nc"]

    in_maps = [_prep_core_inputs(x[c], params, H_DIM, IN_CHAN)
               for c in range(B)]
    res = bass_utils.run_bass_kernel_spmd(nc, in_maps,
                                          core_ids=list(range(N_CORES)))
    bf = params["bf"]
    out = np.stack([res.results[c]["out"].T.ravel() for c in range(B)])
    return (out + bf[None, :]).astype(np.float32)

